# revision 3
# baseline (speedup 1.0000x reference)
"""MultiHeadLiftLayer Trainium2 kernel, v2: edge-pairing gather.

Baseline machinery (pair-packed f16 projection table in SBUF, GPSIMD
ap_gather, DVE parity select, per-128-slot PE matmul with a fixed 0/1
selector), plus host-side edge pairing: the table's node order per core and
per side is a free host choice, so two edges whose src nodes share a packed
u32 (and whose tgt nodes share one too) are served by ONE gather index.
Each index yields two output columns; a per-column parity mask selects which
f16 of the gathered u32 each column takes.

Pairing passes: (A) same-src edges pair (src key = any key holding the node;
both columns take the same parity), (B) same-tgt among leftovers, then the
rest ride as singles (one index, one live column). Index count per side
drops from E to E - #pairs (~0.64E).
"""
import sys

sys.path.insert(0, "/opt/trn_rl_repo")

import numpy as np
import concourse.bass as bass
import concourse.tile as tile
from concourse import bacc, mybir
from concourse.bass_utils import run_bass_kernel_spmd

NUM_NODES = 50000
IN_CH0 = 128
HEADS = 8
NUM_EDGES = 625000
IN_CH1 = 64
OUT_CH = HEADS + IN_CH1  # 72

N_CORES = 8
E_CORE = NUM_EDGES // N_CORES  # 78125
L = 1024                        # gather indices per call per Q7 group
COLS = 2 * L                    # output columns per chunk per call
SLOTS_CALL = 4 * COLS           # 8192 slots per call
NB = COLS // 128                # 16 psum blocks
NT = 2000                       # f16 cols per phase-1 tile
PCHUNK = 500
KEY_CAP = 32768

_cache = {}


def _match_core(src, tgt):
    """Returns (units, m_s, m_t, par_s, par_t, pairs_s, pairs_t).
    units: list of (eA, eB) with eB=-1 for singles. m_s/m_t: per-unit table
    index per side. par_s/par_t: per-unit (parA, parB) column parities.
    pairs_s/pairs_t: [K, 2] node contents of each table pair (-1 = hole).
    """
    E = len(src)
    used = np.zeros(E, bool)
    units = []

    by = {}
    for e in range(E):
        by.setdefault(src[e], []).append(e)
    pairsA = []  # same-src units
    for s, es in by.items():
        while len(es) >= 2:
            e1, e2 = es.pop(), es.pop()
            used[e1] = used[e2] = True
            pairsA.append((e1, e2))
    by = {}
    for e in range(E):
        if not used[e]:
            by.setdefault(tgt[e], []).append(e)
    pairsB = []  # same-tgt units
    for t, es in by.items():
        while len(es) >= 2:
            e1, e2 = es.pop(), es.pop()
            used[e1] = used[e2] = True
            pairsB.append((e1, e2))
    singles = [e for e in range(E) if not used[e]]

    # --- key allocation per side ---
    class Side:
        def __init__(self):
            self.pairs = []      # list of [a, b]
            self.key = {}        # unordered frozen pair -> idx
            self.slot = {}       # node -> (key_idx, parity)

        def alloc_pair(self, a, b):
            k = (a, b) if a <= b else (b, a)
            i = self.key.get(k)
            if i is None:
                i = len(self.pairs)
                self.pairs.append([k[0], k[1]])
                self.key[k] = i
                self.slot.setdefault(k[0], (i, 0))
                self.slot.setdefault(k[1], (i, 1))
            return i

        def need(self, n, pend):
            if n not in self.slot and n not in pend:
                pend[n] = True

        def pack(self, pend):
            ns = [n for n in pend if n not in self.slot]
            for i in range(0, len(ns) - 1, 2):
                self.alloc_pair(ns[i], ns[i + 1])
            if len(ns) % 2:
                self.alloc_pair(ns[-1], ns[-1])

    S, T = Side(), Side()
    # explicit pair keys
    for e1, e2 in pairsA:
        T.alloc_pair(tgt[e1], tgt[e2])
    for e1, e2 in pairsB:
        S.alloc_pair(src[e1], src[e2])
    # packing for membership-only needs
    pend_s, pend_t = {}, {}
    for e1, e2 in pairsA:
        S.need(src[e1], pend_s)
    for e1, e2 in pairsB:
        T.need(tgt[e1], pend_t)
    for e in singles:
        S.need(src[e], pend_s)
        T.need(tgt[e], pend_t)
    S.pack(pend_s)
    T.pack(pend_t)

    m_s, m_t, par_s, par_t = [], [], [], []

    def emit(eA, eB):
        units.append((eA, eB))
        if eB >= 0 and src[eA] == src[eB]:
            i, p = S.slot[src[eA]]
            m_s.append(i)
            par_s.append((p, p))
        elif eB >= 0:
            i = S.alloc_pair(src[eA], src[eB])
            a, b = S.pairs[i]
            m_s.append(i)
            par_s.append((0 if src[eA] == a else 1, 1 if src[eB] == b else 0))
        else:
            i, p = S.slot[src[eA]]
            m_s.append(i)
            par_s.append((p, 0))
        if eB >= 0 and tgt[eA] == tgt[eB]:
            i, p = T.slot[tgt[eA]]
            m_t.append(i)
            par_t.append((p, p))
        elif eB >= 0:
            i = T.alloc_pair(tgt[eA], tgt[eB])
            a, b = T.pairs[i]
            m_t.append(i)
            par_t.append((0 if tgt[eA] == a else 1, 1 if tgt[eB] == b else 0))
        else:
            i, p = T.slot[tgt[eA]]
            m_t.append(i)
            par_t.append((p, 0))

    for e1, e2 in pairsA:
        emit(e1, e2)
    for e1, e2 in pairsB:
        emit(e1, e2)
    for e in singles:
        emit(e, -1)

    assert len(S.pairs) <= KEY_CAP and len(T.pairs) <= KEY_CAP, \
        (len(S.pairs), len(T.pairs))
    return (units, np.array(m_s), np.array(m_t), np.array(par_s),
            np.array(par_t), np.array(S.pairs), np.array(T.pairs))


def _prep(x_0, adjacency_0, x_1, att_parameter):
    src_all = np.asarray(adjacency_0[0]).astype(np.int64)
    tgt_all = np.asarray(adjacency_0[1]).astype(np.int64)
    x_1 = np.asarray(x_1, dtype=np.float32)
    x0f16 = np.asarray(x_0, dtype=np.float32).astype(np.float16)

    wbig = np.empty((IN_CH0, 128), np.float32)
    for p in range(128):
        half = IN_CH0 * (p >= 64)
        wbig[:, p] = att_parameter[half:half + IN_CH0, p % 8]

    msel = np.zeros((128, 32), np.float32)
    for g in range(4):
        for h in range(8):
            msel[16 * g + h, 8 * g + h] = 1.0
            msel[64 + 16 * g + h, 8 * g + h] = 1.0

    cores = []
    umax = ks_max = kt_max = 0
    for core in range(N_CORES):
        lo = core * E_CORE
        r = _match_core(src_all[lo:lo + E_CORE], tgt_all[lo:lo + E_CORE])
        cores.append(r)
        umax = max(umax, len(r[0]))
        ks_max = max(ks_max, len(r[5]))
        kt_max = max(kt_max, len(r[6]))
    ncalls = -(-umax // (4 * L))
    s_core = ncalls * SLOTS_CALL
    # per-side table sizes, rounded so 2*npair_* is a multiple of NT
    npair_s = min(KEY_CAP, -(-ks_max // (NT // 2)) * (NT // 2))
    npair_t = min(KEY_CAP, -(-kt_max // (NT // 2)) * (NT // 2))
    npair = max(npair_s, npair_t)

    in_maps, slot_maps = [], []
    for core in range(N_CORES):
        lo = core * E_CORE
        units, m_s, m_t, par_s, par_t, pairs_s, pairs_t = cores[core]
        U = len(units)

        halves = []
        for np_side, pr in ((npair_s, pairs_s), (npair_t, pairs_t)):
            h = np.zeros((128, np_side, 2), np.float16)
            a, b = pr[:, 0], pr[:, 1]
            n = len(a)
            h[:, :n, 0][:, a >= 0] = x0f16[a[a >= 0]].T
            h[:, :n, 1][:, b >= 0] = x0f16[b[b >= 0]].T
            halves.append(h.reshape(128, 2 * np_side))
        x0t = np.ascontiguousarray(np.concatenate(halves, axis=1))

        grid = 4 * L * ncalls
        mseq = np.zeros(grid, np.int64)
        mtseq = np.zeros(grid, np.int64)
        mseq[:U] = m_s
        mtseq[:U] = m_t
        # masks: [ncalls, 128, COLS] u8; group g rows 16g..16g+16 share the
        # chunk's per-column parity (src side), rows 64+16g.. the tgt side
        pseq = np.zeros((grid, 2), np.uint8)
        tseq = np.zeros((grid, 2), np.uint8)
        pseq[:U] = par_s
        tseq[:U] = par_t
        idx_a = np.zeros((ncalls, 128, L // 16), np.int16)
        mask_a = np.zeros((ncalls, 128, COLS), np.uint8)
        for k in range(ncalls):
            for g in range(4):
                b0 = (k * 4 + g) * L
                w_s = mseq[b0:b0 + L].reshape(L // 16, 16).T.astype(np.int16)
                w_t = mtseq[b0:b0 + L].reshape(L // 16, 16).T.astype(np.int16)
                idx_a[k, 16 * g:16 * g + 16, :] = w_s
                idx_a[k, 64 + 16 * g:64 + 16 * g + 16, :] = w_t
                # column c = 2j + r ; stream position j wraps as 16s+i -> but
                # columns are consumed j-major: unit at stream pos j covers
                # cols 2j, 2j+1 with j = 16*s + i?  No: ap_gather output col
                # j holds index stream position j = 16*s + i where
                # idx[16g+i, s]. Masks are per OUTPUT column: unit u sits at
                # output position j_out = its position in the L-stream.
                ms = pseq[b0:b0 + L].reshape(L, 2)   # unit j -> (parA, parB)
                mt_ = tseq[b0:b0 + L].reshape(L, 2)
                cols_s = np.empty(COLS, np.uint8)
                cols_t = np.empty(COLS, np.uint8)
                cols_s[0::2] = ms[:, 0]
                cols_s[1::2] = ms[:, 1]
                cols_t[0::2] = mt_[:, 0]
                cols_t[1::2] = mt_[:, 1]
                mask_a[k, 16 * g:16 * g + 16, :] = cols_s[None, :]
                mask_a[k, 64 + 16 * g:64 + 16 * g + 16, :] = cols_t[None, :]

        # unit u -> stream (k, g, j): u = (k*4 + g)*L + j  (j-major fill)
        u_ids = np.arange(U)
        k_arr = u_ids // (4 * L)
        g_arr = (u_ids // L) % 4
        j_arr = u_ids % L
        slot_of_edge = np.full(E_CORE, -1, np.int64)
        eA = np.array([u[0] for u in units])
        eB = np.array([u[1] for u in units])
        for par, ee in ((0, eA), (1, eB)):
            c = 2 * j_arr + par
            b = c // 128
            p = c % 128
            slot = SLOTS_CALL * k_arr + 64 * p + 16 * g_arr + b
            v = ee >= 0
            slot_of_edge[ee[v]] = slot[v]
        assert (slot_of_edge >= 0).all()

        x1s = np.zeros((s_core, IN_CH1), np.float32)
        x1s[slot_of_edge] = x_1[lo:lo + E_CORE]

        in_maps.append({
            "x0t": x0t,
            "wbig": wbig,
            "msel": msel,
            "x1": x1s,
            "idx": idx_a,
            "mask": mask_a,
        })
        slot_maps.append(slot_of_edge)
    return in_maps, slot_maps, (npair_s, npair_t), ncalls, s_core


def _build_program(npairs, ncalls, s_core):
    npair_s, npair_t = npairs
    npair = max(npair_s, npair_t)
    key = ("nc", npair_s, npair_t, ncalls)
    if key in _cache:
        return _cache[key]
    nc = bacc.Bacc("TRN2", target_bir_lowering=False, debug=False,
                   num_devices=N_CORES)
    f32, f16, i16 = mybir.dt.float32, mybir.dt.float16, mybir.dt.int16
    u8 = mybir.dt.uint8

    x0t = nc.dram_tensor("x0t", [IN_CH0, 2 * (npair_s + npair_t)], f16,
                         kind="ExternalInput").ap()
    wbig = nc.dram_tensor("wbig", [IN_CH0, 128], f32, kind="ExternalInput").ap()
    x1 = nc.dram_tensor("x1", [s_core, IN_CH1], f32, kind="ExternalInput").ap()
    idx_in = nc.dram_tensor("idx", [ncalls, 128, L // 16], i16,
                            kind="ExternalInput").ap()
    mask_in = nc.dram_tensor("mask", [ncalls, 128, COLS], u8,
                             kind="ExternalInput").ap()
    msel_in = nc.dram_tensor("msel", [128, 32], f32, kind="ExternalInput").ap()
    out = nc.dram_tensor("out", [s_core, OUT_CH], f32, kind="ExternalOutput").ap()

    with tile.TileContext(nc) as tc:
        with tc.tile_pool(name="tab", bufs=1) as tab_pool, \
             tc.tile_pool(name="const", bufs=1) as const_pool:
            tab = tab_pool.tile([128, npair], f32)
            tab_f16 = tab[:].bitcast(f16)            # [128, 2*npair]
            msel32 = const_pool.tile([128, 32], f32)
            nc.sync.dma_start(msel32[:], msel_in[:])
            mselt = const_pool.tile([128, 32], f16)
            nc.vector.tensor_copy(mselt[:], msel32[:])

            # ---- phase 1: both halves of the projection table ----
            # src table cols [0, 2*npair_s), tgt cols [0, 2*npair_t);
            # both 2*npair_* are multiples of NT, so tiles are cleanly
            # "both sides" or "tgt only" (npair_t >= npair_s here).
            ncs, nct = 2 * npair_s, 2 * npair_t
            with tc.tile_pool(name="p1", bufs=4) as p1_pool, \
                 tc.tile_pool(name="p1w", bufs=1) as p1w_pool, \
                 tc.tile_pool(name="p1ps", bufs=6, space="PSUM") as p1ps:
                wb32 = p1w_pool.tile([128, 128], f32)
                nc.sync.dma_start(wb32[:], wbig[:])
                wb16 = p1w_pool.tile([128, 128], f16)
                nc.vector.tensor_copy(wb16[:], wb32[:])
                for t in range(-(-max(ncs, nct) // NT)):
                    c0 = t * NT
                    do_s = c0 < ncs
                    do_t = c0 < nct
                    if do_s:
                        xs = p1_pool.tile([128, NT], f16, tag="xs")
                        nc.sync.dma_start(xs[:], x0t[:, c0:c0 + NT])
                    if do_t:
                        xt = p1_pool.tile([128, NT], f16, tag="xt")
                        nc.sync.dma_start(xt[:], x0t[:, ncs + c0:ncs + c0 + NT])
                    for c in range(NT // PCHUNK):
                        ps = p1ps.tile([128, PCHUNK], f32)
                        if do_s:
                            nc.tensor.matmul(ps[0:64, :], lhsT=wb16[:, 0:64],
                                             rhs=xs[:, c * PCHUNK:(c + 1) * PCHUNK],
                                             start=True, stop=True)
                        if do_t:
                            nc.tensor.matmul(ps[64:128, :], lhsT=wb16[:, 64:128],
                                             rhs=xt[:, c * PCHUNK:(c + 1) * PCHUNK],
                                             start=True, stop=True)
                        dst0 = c0 + c * PCHUNK
                        eng = nc.vector.tensor_copy if c % 2 == 0 else nc.scalar.copy
                        if do_s:
                            eng(tab_f16[:, dst0:dst0 + PCHUNK], ps[:])
                        else:
                            eng(tab_f16[64:128, dst0:dst0 + PCHUNK], ps[64:128, :])

            # ---- phase 2 ----
            with tc.tile_pool(name="io", bufs=2) as io_pool, \
                 tc.tile_pool(name="idxp", bufs=1) as idx_pool, \
                 tc.tile_pool(name="mega", bufs=2) as mega_pool, \
                 tc.tile_pool(name="p2ps", bufs=2, space="PSUM") as p2ps:
                its = []
                for k in range(ncalls):
                    it = idx_pool.tile([128, L // 16], i16, tag=f"it{k}")
                    nc.sync.dma_start(it[:], idx_in[k])
                    its.append(it)
                for k in range(ncalls):
                    mk = io_pool.tile([128, COLS], u8, tag="mk")
                    nc.sync.dma_start(mk[:], mask_in[k])
                    ot = io_pool.tile([128, L], f32, tag="ot")
                    nc.gpsimd.ap_gather(out_ap=ot[:], in_ap=tab[:],
                                        idxs_ap=its[k][:], channels=128,
                                        num_elems=npair, d=1, num_idxs=L)
                    pair = ot[:].bitcast(f16).rearrange(
                        "p (l two) -> p l two", two=2)
                    sel = io_pool.tile([128, COLS], f16, tag="sel")
                    selv = sel[:].rearrange("p (l two) -> p l two", two=2)
                    mkv = mk[:].rearrange("p (l two) -> p l two", two=2)
                    for r in range(2):
                        nc.vector.tensor_copy(selv[:, :, r], pair[:, :, 0])
                        nc.vector.copy_predicated(selv[:, :, r], mkv[:, :, r],
                                                  pair[:, :, 1])

                    ps2 = p2ps.tile([128, 512], f32)
                    for b in range(NB):
                        nc.tensor.matmul(ps2[:, 32 * b:32 * b + 32],
                                         lhsT=sel[:, 128 * b:128 * (b + 1)],
                                         rhs=mselt[:], start=True, stop=True)

                    mega = mega_pool.tile([128, 64, OUT_CH], f32)
                    psv = ps2[:].rearrange("p (s h) -> p s h", h=32)
                    for g in range(4):
                        nc.scalar.activation(
                            mega[:, g * NB:(g + 1) * NB, :HEADS],
                            psv[:, :, 8 * g:8 * g + 8],
                            mybir.ActivationFunctionType.Relu)
                    v = slice(k * SLOTS_CALL, (k + 1) * SLOTS_CALL)
                    nc.sync.dma_start(
                        mega[:, :, HEADS:],
                        x1[v].rearrange("(p s) c -> p s c", s=64))
                    nc.scalar.dma_start(
                        out[v].rearrange("(p s) c -> p s c", s=64),
                        mega[:])

    nc.compile()
    _cache[key] = nc
    return nc


def kernel(x_0, adjacency_0, x_1, att_parameter, _trace=False):
    x_0 = np.asarray(x_0, dtype=np.float32)
    adjacency_0 = np.asarray(adjacency_0)
    x_1 = np.asarray(x_1, dtype=np.float32)
    att_parameter = np.asarray(att_parameter, dtype=np.float32)
    in_maps, slot_maps, npair, ncalls, s_core = _prep(
        x_0, adjacency_0, x_1, att_parameter)
    nc = _build_program(npair, ncalls, s_core)
    res = run_bass_kernel_spmd(nc, in_maps, list(range(N_CORES)), trace=_trace)
    outs = []
    for core in range(N_CORES):
        outs.append(res.results[core]["out"][slot_maps[core]])
    kernel.last_exec_time_ns = res.exec_time_ns
    return np.concatenate(outs, axis=0)
